# revision 1
# baseline (speedup 1.0000x reference)
"""GNN message-passing layer (LplsNorm + residual conv) on 8 Trainium2 cores.

Computation (reference, all f32):
    degree = A.sum(-1); ds = degree**-0.5
    mf  = f + ds[:,None] * (A @ (ds[:,None] * f))      # a_norm = ds A ds
    out = relu(mf @ W + b)

Distribution: A row-sharded over 8 cores ([1024, 8192] each), feature
replicated.

Per-core schedule (v3):
  - Single streaming pass over the A shard. Per [128, 2048] chunk: ScalarE
    accumulates row sums (degree, f32-exact), GpSimd casts the chunk to
    bf16, TensorE transposes the 16 [128,128] bf16 tiles (cheap LDW), DVE
    copies them out of PSUM. 7/16ths stay resident in SBUF; the rest spill
    to a bf16 DRAM scratch. Keeps the PE warm and off the f32 weight-load
    path.
  - Tiny AllGather shares per-core degree; ds = 1/sqrt(degree) via DVE
    reciprocal + ACT sqrt.
  - X' = ds * f cast to bf16, produced in 1 MiB batches.
  - Main matmul runs kc-outer over groups of 4 m-tiles (4 PSUM
    accumulators), so the PE saturates as soon as the first X' chunks
    appear instead of being paced by X' production.
  - Epilogue per m-tile: mf = Y * ds_own + f_res (fused DVE op), mf @ W in
    f32r (full-rate PE), bias via a K=1 matmul with a ones row, ACT relu.
"""

import numpy as np

import concourse.bass as bass
import concourse.mybir as mybir
import concourse.tile as tile
from concourse import bacc
from concourse import bass_utils
from concourse.masks import make_identity

N = 8192
D = 512
NCORES = 8
P = 128
R = N // NCORES          # rows per core: 1024
MT = R // P              # m-tiles per core: 8
KC = N // P              # k-chunks: 64
ACH = 2048               # A stream chunk width (f32 -> 1 MiB per DMA)
NACH = N // ACH          # stream chunks per row-block: 4
GPC = ACH // (4 * P)     # transpose groups (of 4 tiles) per stream chunk: 4
NG = KC // 4             # k-groups total: 16
NG_RES = 7               # k-groups resident in SBUF (kc 0..27)
MTG = 4                  # m-tiles per matmul group (PSUM accumulators)

F32 = mybir.dt.float32
F32R = mybir.dt.float32r
BF16 = mybir.dt.bfloat16

_NC_CACHE = {}


def _build():
    nc = bacc.Bacc("TRN2", target_bir_lowering=False, debug=False, num_devices=NCORES)

    a_d = nc.dram_tensor("a", [R, N], F32, kind="ExternalInput")
    f_d = nc.dram_tensor("f", [N, D], F32, kind="ExternalInput")
    fres_d = nc.dram_tensor("fres", [R, D], F32, kind="ExternalInput")
    w_d = nc.dram_tensor("w", [D, D], F32R, kind="ExternalInput")
    b_d = nc.dram_tensor("bias", [1, D], F32, kind="ExternalInput")
    out_d = nc.dram_tensor("out", [R, D], F32, kind="ExternalOutput")

    AX = mybir.AxisListType.X
    ALU = mybir.AluOpType
    ACT = mybir.ActivationFunctionType

    with tile.TileContext(nc) as tc:
        with (
            tc.tile_pool(name="const", bufs=1) as constp,
            tc.tile_pool(name="deg", bufs=1) as degp,
            tc.tile_pool(name="astream", bufs=3) as astreamp,
            tc.tile_pool(name="small", bufs=2) as smallp,
            tc.tile_pool(name="atres", bufs=1) as atresp,
            tc.tile_pool(name="atw", bufs=2) as atwp,
            tc.tile_pool(name="xp", bufs=1) as xpp,
            tc.tile_pool(name="fstream", bufs=2) as fstreamp,
            tc.tile_pool(name="epi", bufs=2) as epip,
            tc.tile_pool(name="mft", bufs=2) as mftp,
            tc.tile_pool(name="psA", bufs=2, space="PSUM") as psA,      # transpose groups
            tc.tile_pool(name="psY", bufs=MTG, space="PSUM") as psY,    # Y accumulators
            tc.tile_pool(name="psaux", bufs=1, space="PSUM") as psaux,  # small transposes
            tc.tile_pool(name="psO", bufs=1, space="PSUM") as psO,      # second matmul out
            tc.tile_pool(name="dram", bufs=1, space="DRAM") as dramp,
        ):
            # ---- constants ----
            identity = constp.tile([P, P], F32)
            make_identity(nc, identity[:])
            identity_bf = constp.tile([P, P], BF16)
            make_identity(nc, identity_bf[:])
            ones_row = constp.tile([1, P], F32)
            nc.gpsimd.memset(ones_row[:], 1.0)
            b_sb = constp.tile([1, D], F32)
            nc.sync.dma_start(b_sb[:], b_d.ap())
            w_sb = constp.tile([P, 4 * D], F32R)  # w chunk wc at [:, wc*D:(wc+1)*D]
            for wc in range(4):
                nc.sync.dma_start(
                    w_sb[:, wc * D : (wc + 1) * D], w_d.ap()[wc * P : (wc + 1) * P, :]
                )

            # resident transposed-A store: (group g, mt) block at col (g*MT+mt)*4P
            at_res = atresp.tile([P, NG_RES * MT * 4 * P], BF16)
            # DRAM scratch for the non-resident groups
            scratch = dramp.tile([(NG - NG_RES) * MT, P, 4 * P], BF16)
            cin = dramp.tile([MT, P], F32)
            cout = dramp.tile([KC, P], F32)

            # ---- merged pass: degree + transpose-all ----
            degree_sb = degp.tile([P, MT], F32)  # col mt = degree of rows mt*128..
            for mt in range(MT):
                dcols = smallp.tile([P, NACH], F32, tag="dcols")
                for c in range(NACH):
                    ach = astreamp.tile([P, ACH], F32, tag="ach")
                    nc.sync.dma_start(
                        ach[:], a_d.ap()[mt * P : (mt + 1) * P, c * ACH : (c + 1) * ACH]
                    )
                    achb = astreamp.tile([P, ACH], BF16, tag="achb", bufs=2)
                    nc.scalar.activation(
                        achb[:], ach[:], ACT.Copy, accum_out=dcols[:, c : c + 1]
                    )
                    for g in range(GPC):
                        gk = c * GPC + g  # k-group index 0..15
                        trp = psA.tile([P, 4 * P], F32, tag="trp")
                        for q in range(4):
                            nc.tensor.matmul(
                                trp[:, q * P : (q + 1) * P],
                                achb[:, (g * 4 + q) * P : (g * 4 + q + 1) * P],
                                identity_bf[:],
                            )
                        if gk < NG_RES:
                            dst = at_res[
                                :, (gk * MT + mt) * 4 * P : (gk * MT + mt + 1) * 4 * P
                            ]
                        else:
                            dst = atwp.tile([P, 4 * P], BF16, tag="atw")
                        nc.vector.tensor_copy(dst[:], trp[:])
                        if gk >= NG_RES:
                            nc.sync.dma_start(
                                scratch[(gk - NG_RES) * MT + mt], dst[:]
                            )
                nc.vector.reduce_sum(degree_sb[:, mt : mt + 1], dcols[:], axis=AX)

            # ---- AllGather degree ----
            degT_ps = psaux.tile([MT, P], F32, tag="aux")
            nc.tensor.transpose(degT_ps[:], degree_sb[:], identity[:])
            degT_sb = smallp.tile([MT, P], F32, tag="degT")
            nc.vector.tensor_copy(degT_sb[:], degT_ps[:])
            nc.sync.dma_start(cin[:], degT_sb[:])
            nc.gpsimd.collective_compute(
                "AllGather",
                ALU.bypass,
                ins=[cin.opt()],
                outs=[cout.opt()],
                replica_groups=[list(range(NCORES))],
            )
            # cout row g = global degree of rows [g*128, (g+1)*128)
            degall_sb = smallp.tile([KC, P], F32, tag="degall")
            nc.sync.dma_start(degall_sb[:], cout[:])
            degallT_ps = psaux.tile([P, KC], F32, tag="aux")
            nc.tensor.transpose(degallT_ps[:], degall_sb[:], identity[:KC, :KC])
            recip = degp.tile([P, KC], F32)
            nc.vector.reciprocal(recip[:], degallT_ps[:])
            ds_sb = degp.tile([P, KC], F32)  # ds_sb[p, g] = ds[g*128 + p]
            nc.scalar.activation(ds_sb[:], recip[:], ACT.Sqrt)
            recip8 = degp.tile([P, MT], F32)
            nc.vector.reciprocal(recip8[:], degree_sb[:])
            dsown = degp.tile([P, MT], F32)
            nc.scalar.activation(dsown[:], recip8[:], ACT.Sqrt)

            # ---- X' = ds * f, cast to bf16 (1 MiB load batches) ----
            xp_sb = xpp.tile([P, KC * D], BF16)  # chunk kc at [:, kc*D:(kc+1)*D]
            f_blk = f_d.ap().rearrange("(b c p) d -> b p c d", c=4, p=P)
            for fb in range(KC // 4):
                fch = fstreamp.tile([P, 4 * D], F32, tag="fch")
                nc.sync.dma_start(
                    fch[:].rearrange("p (c d) -> p c d", c=4), f_blk[fb]
                )
                for j in range(4):
                    kc = 4 * fb + j
                    nc.vector.tensor_scalar_mul(
                        xp_sb[:, kc * D : (kc + 1) * D],
                        fch[:, j * D : (j + 1) * D],
                        ds_sb[:, kc : kc + 1],
                    )

            # ---- main matmul: kc-outer over groups of MTG m-tiles ----
            for mtg in range(MT // MTG):
                ys = [psY.tile([P, D], F32, tag="y", name=f"y{mtg}_{i}") for i in range(MTG)]
                for gk in range(NG):
                    at4s = []
                    for mi in range(MTG):
                        mt = mtg * MTG + mi
                        if gk < NG_RES:
                            at4 = at_res[
                                :, (gk * MT + mt) * 4 * P : (gk * MT + mt + 1) * 4 * P
                            ]
                        else:
                            at4t = atwp.tile([P, 4 * P], BF16, tag="atr", bufs=6)
                            nc.sync.dma_start(
                                at4t[:], scratch[(gk - NG_RES) * MT + mt]
                            )
                            at4 = at4t[:]
                        at4s.append(at4)
                    for q in range(4):
                        kc = gk * 4 + q
                        for mi in range(MTG):
                            nc.tensor.matmul(
                                ys[mi][:],
                                at4s[mi][:, q * P : (q + 1) * P],
                                xp_sb[:, kc * D : (kc + 1) * D],
                                start=(kc == 0),
                                stop=(kc == KC - 1),
                            )
                # epilogue per m-tile in the group
                for mi in range(MTG):
                    mt = mtg * MTG + mi
                    res = epip.tile([P, D], F32, tag="res")
                    nc.sync.dma_start(res[:], fres_d.ap()[mt * P : (mt + 1) * P, :])
                    mf = epip.tile([P, D], F32, tag="mf")
                    nc.vector.scalar_tensor_tensor(
                        mf[:],
                        ys[mi][:],
                        dsown[:, mt : mt + 1],
                        res[:],
                        op0=ALU.mult,
                        op1=ALU.add,
                    )
                    o_ps = psO.tile([P, D], F32, tag="o")
                    for wc in range(4):
                        mfT_ps = psaux.tile([P, P], F32, tag="aux")
                        nc.tensor.transpose(
                            mfT_ps[:], mf[:, wc * P : (wc + 1) * P], identity[:]
                        )
                        mfT_sb = mftp.tile([P, P], F32R, tag="mfT")
                        nc.vector.tensor_copy(mfT_sb[:], mfT_ps[:])
                        nc.tensor.matmul(
                            o_ps[:],
                            mfT_sb[:],
                            w_sb[:, wc * D : (wc + 1) * D],
                            start=(wc == 0),
                            stop=False,
                        )
                    nc.tensor.matmul(
                        o_ps[:], ones_row[:], b_sb[:], start=False, stop=True
                    )
                    osb = epip.tile([P, D], F32, tag="osb")
                    nc.scalar.activation(osb[:], o_ps[:], ACT.Relu)
                    nc.sync.dma_start(out_d.ap()[mt * P : (mt + 1) * P, :], osb[:])

    nc.compile()
    return nc


def _get_nc():
    if "nc" not in _NC_CACHE:
        _NC_CACHE["nc"] = _build()
    return _NC_CACHE["nc"]


def run(inputs, trace=False, trace_kwargs=None):
    """Run the SPMD kernel; returns (full_output, BassKernelResults)."""
    a = np.ascontiguousarray(np.asarray(inputs["adjacency_matrix"], dtype=np.float32))
    f = np.ascontiguousarray(np.asarray(inputs["feature"], dtype=np.float32))
    w = np.ascontiguousarray(np.asarray(inputs["W"], dtype=np.float32))
    b = np.ascontiguousarray(np.asarray(inputs["b"], dtype=np.float32)).reshape(1, D)

    nc = _get_nc()
    in_maps = []
    for d in range(NCORES):
        rows = slice(d * R, (d + 1) * R)
        in_maps.append({"a": a[rows], "f": f, "fres": f[rows], "w": w, "bias": b})
    res = bass_utils.run_bass_kernel_spmd(
        nc,
        in_maps,
        core_ids=list(range(NCORES)),
        trace=trace,
        **(trace_kwargs or {}),
    )
    out = np.concatenate([r["out"] for r in res.results], axis=0)
    return out, res


def kernel(**inputs):
    out, _ = run(inputs, trace=False)
    return out



# revision 3
# speedup vs baseline: 1.4912x; 1.4912x over previous
"""GNN message-passing layer (LplsNorm + residual conv) on 8 Trainium2 cores.

Computation (reference, all f32):
    degree = A.sum(-1); ds = degree**-0.5
    mf  = f + ds[:,None] * (A @ (ds[:,None] * f))      # a_norm = ds A ds
    out = relu(mf @ W + b)
Distribution: A row-sharded over 8 cores ([1024, 8192] each), feature
replicated.

v4 schedule (fp8 + DoubleRow, no DRAM spill):
  - Phase 1: stream the A shard once. ScalarE casts each [128,2048] chunk
    to fp8-e4m3 while accumulating exact f32 row sums (degree); TensorE
    transposes the fp8 tiles (matmul vs fp8 identity); DVE copies them to
    a fully SBUF-resident transposed-A store (8 MiB fp8 -> no spill).
  - The A@X term contributes only ~0.7% of mf (the residual f dominates),
    so fp8 for A and X keeps total l2 rel err ~4e-4 (verified vs numpy).
  - Tiny AllGather shares per-core degree. X'' = (64*ds)*f cast to fp8
    (scaled into fp8's happy range; un-scaled by dsown/64 in the
    epilogue). f streams during phase 2 when DMA is otherwise idle.
  - Main matmul: DoubleRow fp8 pairs two k-chunks per instruction
    (0.5 cyc/row), kc-outer over groups of 4 m-tiles (4 PSUM banks).
  - Epilogue per m-tile: mf = Y * dsown/64 + f_res (fused DVE op),
    mf @ W in f32r, bias via a K=1 matmul with a ones row, ACT relu.
"""

import numpy as np

import concourse.bass as bass
import concourse.mybir as mybir
import concourse.tile as tile
from concourse import bacc
from concourse import bass_utils
from concourse.masks import make_identity

N = 8192
D = 512
NCORES = 8
P = 128
R = N // NCORES          # rows per core: 1024
MT = R // P              # m-tiles per core: 8
KC = N // P              # k-chunks: 64
ACH = 2048               # A stream chunk width (f32 -> 1 MiB per DMA)
NACH = N // ACH          # stream chunks per row-block: 4
GPC = ACH // (4 * P)     # transpose groups (of 4 tiles) per stream chunk: 4
NG = KC // 4             # k-groups total: 16
MTG = 4                  # m-tiles per matmul group (PSUM accumulators)
NPAIR = KC // 2          # DoubleRow k-chunk pairs: 32

F32 = mybir.dt.float32
F32R = mybir.dt.float32r
F8 = mybir.dt.float8e4

_NC_CACHE = {}


def _build():
    nc = bacc.Bacc("TRN2", target_bir_lowering=False, debug=False, num_devices=NCORES)

    a_d = nc.dram_tensor("a", [R, N], F32, kind="ExternalInput")
    f_d = nc.dram_tensor("f", [N, D], F32, kind="ExternalInput")
    fres_d = nc.dram_tensor("fres", [R, D], F32, kind="ExternalInput")
    w_d = nc.dram_tensor("w", [D, D], F32R, kind="ExternalInput")
    b_d = nc.dram_tensor("bias", [1, D], F32, kind="ExternalInput")
    out_d = nc.dram_tensor("out", [R, D], F32, kind="ExternalOutput")

    AX = mybir.AxisListType.X
    ALU = mybir.AluOpType
    ACT = mybir.ActivationFunctionType
    DR = mybir.MatmulPerfMode.DoubleRow

    with tile.TileContext(nc) as tc:
        with (
            tc.tile_pool(name="const", bufs=1) as constp,
            tc.tile_pool(name="deg", bufs=1) as degp,
            tc.tile_pool(name="astream", bufs=3) as astreamp,
            tc.tile_pool(name="small", bufs=2) as smallp,
            tc.tile_pool(name="atres", bufs=1) as atresp,
            tc.tile_pool(name="xp", bufs=1) as xpp,
            tc.tile_pool(name="fstream", bufs=3) as fstreamp,
            tc.tile_pool(name="epi", bufs=2) as epip,
            tc.tile_pool(name="mft", bufs=2) as mftp,
            tc.tile_pool(name="psA", bufs=2, space="PSUM") as psA,      # transpose groups
            tc.tile_pool(name="psY", bufs=MTG, space="PSUM") as psY,    # Y accumulators
            tc.tile_pool(name="psaux", bufs=1, space="PSUM") as psaux,  # small transposes
            tc.tile_pool(name="psO", bufs=1, space="PSUM") as psO,      # second matmul out
            tc.tile_pool(name="dram", bufs=1, space="DRAM") as dramp,
        ):
            # ---- constants ----
            identity = constp.tile([P, P], F32)
            make_identity(nc, identity[:])
            identity_f8 = constp.tile([P, P], F8)
            make_identity(nc, identity_f8[:])
            ones_row = constp.tile([1, P], F32)
            nc.gpsimd.memset(ones_row[:], 1.0)
            b_sb = constp.tile([1, D], F32)
            nc.sync.dma_start(b_sb[:], b_d.ap())
            w_sb = constp.tile([P, 4 * D], F32R)  # w chunk wc at [:, wc*D:(wc+1)*D]
            for wc in range(4):
                nc.sync.dma_start(
                    w_sb[:, wc * D : (wc + 1) * D], w_d.ap()[wc * P : (wc + 1) * P, :]
                )

            # resident transposed-A store: (mt, kc) tile at col (mt*KC+kc)*P
            at_res = atresp.tile([P, MT * KC * P], F8)
            cin = dramp.tile([MT, P], F32)
            cout = dramp.tile([KC, P], F32)

            # ---- merged pass: degree + transpose-all ----
            degree_sb = degp.tile([P, MT], F32)  # col mt = degree of rows mt*128..
            for mt in range(MT):
                dcols = smallp.tile([P, NACH], F32, tag="dcols")
                for c in range(NACH):
                    ach = astreamp.tile([P, ACH], F32, tag="ach")
                    nc.sync.dma_start(
                        ach[:], a_d.ap()[mt * P : (mt + 1) * P, c * ACH : (c + 1) * ACH]
                    )
                    ach8 = astreamp.tile([P, ACH], F8, tag="ach8", bufs=2)
                    nc.scalar.activation(
                        ach8[:], ach[:], ACT.Copy, accum_out=dcols[:, c : c + 1]
                    )
                    for g in range(GPC):
                        gk = c * GPC + g  # k-group index 0..15
                        trp = psA.tile([P, 4 * P], F32, tag="trp")
                        for q in range(4):
                            nc.tensor.matmul(
                                trp[:, q * P : (q + 1) * P],
                                ach8[:, (g * 4 + q) * P : (g * 4 + q + 1) * P],
                                identity_f8[:],
                            )
                        dst = at_res[
                            :, (mt * KC + gk * 4) * P : (mt * KC + gk * 4 + 4) * P
                        ]
                        nc.vector.tensor_copy(dst, trp[:])
                nc.vector.reduce_sum(degree_sb[:, mt : mt + 1], dcols[:], axis=AX)

            # ---- AllGather degree ----
            degT_ps = psaux.tile([MT, P], F32, tag="aux")
            nc.tensor.transpose(degT_ps[:], degree_sb[:], identity[:])
            degT_sb = smallp.tile([MT, P], F32, tag="degT")
            nc.vector.tensor_copy(degT_sb[:], degT_ps[:])
            nc.sync.dma_start(cin[:], degT_sb[:])
            nc.gpsimd.collective_compute(
                "AllGather",
                ALU.bypass,
                ins=[cin.opt()],
                outs=[cout.opt()],
                replica_groups=[list(range(NCORES))],
            )
            # cout row g = global degree of rows [g*128, (g+1)*128)
            degall_sb = smallp.tile([KC, P], F32, tag="degall")
            nc.sync.dma_start(degall_sb[:], cout[:])
            degallT_ps = psaux.tile([P, KC], F32, tag="aux")
            nc.tensor.transpose(degallT_ps[:], degall_sb[:], identity[:KC, :KC])
            recip = degp.tile([P, KC], F32)
            nc.vector.reciprocal(recip[:], degallT_ps[:])
            # ds64_sb[p, g] = 64 / sqrt(degree[g*128 + p])
            ds64_sb = degp.tile([P, KC], F32)
            nc.scalar.activation(ds64_sb[:], recip[:], ACT.Sqrt, scale=4096.0)
            recip8 = degp.tile([P, MT], F32)
            nc.vector.reciprocal(recip8[:], degree_sb[:])
            # dsown64[p, mt] = 1 / (64 * sqrt(degree_own[mt*128 + p]))
            dsown64 = degp.tile([P, MT], F32)
            nc.scalar.activation(dsown64[:], recip8[:], ACT.Sqrt, scale=1.0 / 4096.0)

            # ---- X'' = (64*ds) * f, cast to fp8 (1 MiB load batches) ----
            xp_sb = xpp.tile([P, KC * D], F8)  # chunk kc at [:, kc*D:(kc+1)*D]
            f_blk = f_d.ap().rearrange("(b c p) d -> b p c d", c=4, p=P)
            for fb in range(KC // 4):
                fch = fstreamp.tile([P, 4 * D], F32, tag="fch")
                nc.sync.dma_start(
                    fch[:].rearrange("p (c d) -> p c d", c=4), f_blk[fb]
                )
                for j in range(4):
                    kc = 4 * fb + j
                    nc.vector.tensor_scalar_mul(
                        xp_sb[:, kc * D : (kc + 1) * D],
                        fch[:, j * D : (j + 1) * D],
                        ds64_sb[:, kc : kc + 1],
                    )

            # ---- main matmul: DoubleRow fp8, kc-pair outer over MTG m-tiles ----
            for mtg in range(MT // MTG):
                ys = [psY.tile([P, D], F32, tag="y", name=f"y{mtg}_{i}") for i in range(MTG)]
                for jp in range(NPAIR):
                    xp2 = xp_sb[:, (2 * jp) * D : (2 * jp + 2) * D].rearrange(
                        "p (k n) -> p k n", k=2
                    )
                    for mi in range(MTG):
                        mt = mtg * MTG + mi
                        base = (mt * KC + 2 * jp) * P
                        at2 = at_res[:, base : base + 2 * P].rearrange(
                            "p (k m) -> p k m", k=2
                        )
                        nc.tensor.matmul(
                            ys[mi][:],
                            at2,
                            xp2,
                            start=(jp == 0),
                            stop=(jp == NPAIR - 1),
                            perf_mode=DR,
                        )
                # epilogue per m-tile in the group
                for mi in range(MTG):
                    mt = mtg * MTG + mi
                    res = epip.tile([P, D], F32, tag="res")
                    nc.sync.dma_start(res[:], fres_d.ap()[mt * P : (mt + 1) * P, :])
                    mf = epip.tile([P, D], F32, tag="mf")
                    nc.vector.scalar_tensor_tensor(
                        mf[:],
                        ys[mi][:],
                        dsown64[:, mt : mt + 1],
                        res[:],
                        op0=ALU.mult,
                        op1=ALU.add,
                    )
                    o_ps = psO.tile([P, D], F32, tag="o")
                    for wc in range(4):
                        mfT_ps = psaux.tile([P, P], F32, tag="aux")
                        nc.tensor.transpose(
                            mfT_ps[:], mf[:, wc * P : (wc + 1) * P], identity[:]
                        )
                        mfT_sb = mftp.tile([P, P], F32R, tag="mfT")
                        nc.vector.tensor_copy(mfT_sb[:], mfT_ps[:])
                        nc.tensor.matmul(
                            o_ps[:],
                            mfT_sb[:],
                            w_sb[:, wc * D : (wc + 1) * D],
                            start=(wc == 0),
                            stop=False,
                        )
                    nc.tensor.matmul(
                        o_ps[:], ones_row[:], b_sb[:], start=False, stop=True
                    )
                    osb = epip.tile([P, D], F32, tag="osb")
                    nc.scalar.activation(osb[:], o_ps[:], ACT.Relu)
                    nc.sync.dma_start(out_d.ap()[mt * P : (mt + 1) * P, :], osb[:])

    nc.compile()
    return nc


def _get_nc():
    if "nc" not in _NC_CACHE:
        _NC_CACHE["nc"] = _build()
    return _NC_CACHE["nc"]


def run(inputs, trace=False, trace_kwargs=None):
    """Run the SPMD kernel; returns (full_output, BassKernelResults)."""
    a = np.ascontiguousarray(np.asarray(inputs["adjacency_matrix"], dtype=np.float32))
    f = np.ascontiguousarray(np.asarray(inputs["feature"], dtype=np.float32))
    w = np.ascontiguousarray(np.asarray(inputs["W"], dtype=np.float32))
    b = np.ascontiguousarray(np.asarray(inputs["b"], dtype=np.float32)).reshape(1, D)

    nc = _get_nc()
    in_maps = []
    for d in range(NCORES):
        rows = slice(d * R, (d + 1) * R)
        in_maps.append({"a": a[rows], "f": f, "fres": f[rows], "w": w, "bias": b})
    res = bass_utils.run_bass_kernel_spmd(
        nc,
        in_maps,
        core_ids=list(range(NCORES)),
        trace=trace,
        **(trace_kwargs or {}),
    )
    out = np.concatenate([r["out"] for r in res.results], axis=0)
    return out, res


def kernel(**inputs):
    out, _ = run(inputs, trace=False)
    return out


# revision 5
# speedup vs baseline: 1.5177x; 1.0178x over previous
"""GNN message-passing layer (LplsNorm + residual conv) on 8 Trainium2 cores.

Computation (reference, all f32):
    degree = A.sum(-1); ds = degree**-0.5
    mf  = f + ds[:,None] * (A @ (ds[:,None] * f))      # a_norm = ds A ds
    out = relu(mf @ W + b)
Distribution: A row-sharded over 8 cores ([1024, 8192] each), feature
replicated.

v5 schedule (fp8 DoubleRow main matmul, f parked through the collective):
  - Phase 1: stream the A shard once (DMA-bound, ~330 GB/s). ScalarE casts
    each [128,2048] chunk to fp8-e4m3 while accumulating exact f32 row sums
    (degree); TensorE transposes the fp8 tiles; DVE copies them into a fully
    SBUF-resident transposed-A store (8 MiB fp8, no DRAM spill).
  - The A@X term contributes only ~0.7% of mf (the residual f dominates), so
    fp8 there keeps total l2 rel err ~5e-4 (verified vs numpy; gate 2e-2).
  - Degree AllGather's barrier+latency window is covered by streaming f:
    ScalarE parks raw f as fp8 (no ds needed), so the f DMA never stalls on
    the collective. After ds arrives, DVE produces X'' = (64*ds)*f8 chunks
    that the matmul chases.
  - Main matmul: DoubleRow fp8 (2 k-chunks per instruction, 0.5 cyc/row).
    Group 1 covers m-tiles 0-5 in 6 PSUM banks; m-tiles 6-7 + the epilogue's
    o-accumulators rotate through the same 6-bank pool as tiles free up.
    psA's 2 banks serve phase-1 transposes, then the epilogue's mf
    transposes.
  - Epilogue: mf = Y * dsown/64 + f_res (DVE, bf16 out), mf @ W in bf16,
    bias via a K=1 bf16 matmul with a ones row, ACT relu, store.
"""

import numpy as np

import concourse.bass as bass
import concourse.mybir as mybir
import concourse.tile as tile
from concourse import bacc
from concourse import bass_utils
from concourse.masks import make_identity

N = 8192
D = 512
NCORES = 8
P = 128
R = N // NCORES          # rows per core: 1024
MT = R // P              # m-tiles per core: 8
KC = N // P              # k-chunks: 64
ACH = 2048               # A stream chunk width (f32 -> 1 MiB per DMA)
NACH = N // ACH          # stream chunks per row-block: 4
GPC = ACH // (4 * P)     # transpose groups (of 4 tiles) per stream chunk: 4
MTG = 6                  # m-tiles in the big matmul group (PSUM banks)
NPAIR = KC // 2          # DoubleRow k-chunk pairs: 32
FCH = 2                  # f stream batch: 2 k-chunks (512 KiB per DMA)

F32 = mybir.dt.float32
BF16 = mybir.dt.bfloat16
F8 = mybir.dt.float8e4

_NC_CACHE = {}


def _build():
    nc = bacc.Bacc("TRN2", target_bir_lowering=False, debug=False, num_devices=NCORES)

    a_d = nc.dram_tensor("a", [R, N], F32, kind="ExternalInput")
    f_d = nc.dram_tensor("f", [N, D], F32, kind="ExternalInput")
    fres_d = nc.dram_tensor("fres", [R, D], F32, kind="ExternalInput")
    w_d = nc.dram_tensor("w", [D, D], F32, kind="ExternalInput")
    b_d = nc.dram_tensor("bias", [1, D], F32, kind="ExternalInput")
    out_d = nc.dram_tensor("out", [R, D], F32, kind="ExternalOutput")

    AX = mybir.AxisListType.X
    ALU = mybir.AluOpType
    ACT = mybir.ActivationFunctionType
    DR = mybir.MatmulPerfMode.DoubleRow

    with tile.TileContext(nc) as tc:
        with (
            tc.tile_pool(name="const", bufs=1) as constp,
            tc.tile_pool(name="deg", bufs=1) as degp,
            tc.tile_pool(name="astream", bufs=2) as astreamp,
            tc.tile_pool(name="small", bufs=2) as smallp,
            tc.tile_pool(name="atres", bufs=1) as atresp,
            tc.tile_pool(name="xp", bufs=1) as xpp,
            tc.tile_pool(name="f8p", bufs=1) as f8pool,
            tc.tile_pool(name="fstream", bufs=3) as fstreamp,
            tc.tile_pool(name="epi", bufs=2) as epip,
            tc.tile_pool(name="mfp", bufs=MT) as mfpool,
            tc.tile_pool(name="mft", bufs=2) as mftp,
            tc.tile_pool(name="psA", bufs=2, space="PSUM") as psA,      # transposes
            tc.tile_pool(name="psY", bufs=MTG, space="PSUM") as psY,    # Y + o accum
            tc.tile_pool(name="dram", bufs=1, space="DRAM") as dramp,
        ):
            # ---- constants ----
            identity = constp.tile([P, P], F32)
            make_identity(nc, identity[:])
            identity_f8 = constp.tile([P, P], F8)
            make_identity(nc, identity_f8[:])
            identity_bf = constp.tile([P, P], BF16)
            make_identity(nc, identity_bf[:])
            ones_row = constp.tile([1, P], BF16)
            nc.gpsimd.memset(ones_row[:], 1.0)
            bf_sb = constp.tile([1, D], F32)
            nc.sync.dma_start(bf_sb[:], b_d.ap())
            b_sb = constp.tile([1, D], BF16)
            nc.vector.tensor_copy(b_sb[:], bf_sb[:])
            wf_sb = constp.tile([P, 4 * D], F32)  # w chunk wc at [:, wc*D:(wc+1)*D]
            for wc in range(4):
                nc.sync.dma_start(
                    wf_sb[:, wc * D : (wc + 1) * D], w_d.ap()[wc * P : (wc + 1) * P, :]
                )
            w_sb = constp.tile([P, 4 * D], BF16)
            nc.vector.tensor_copy(w_sb[:], wf_sb[:])

            # resident transposed-A store: (mt, kc) tile at col (mt*KC+kc)*P
            at_res = atresp.tile([P, MT * KC * P], F8)
            cin = dramp.tile([MT, P], F32)
            cout = dramp.tile([KC, P], F32)

            # ---- merged pass: degree + transpose-all ----
            degree_sb = degp.tile([P, MT], F32)  # col mt = degree of rows mt*128..
            for mt in range(MT):
                dcols = smallp.tile([P, NACH], F32, tag="dcols")
                for c in range(NACH):
                    ach = astreamp.tile([P, ACH], F32, tag="ach")
                    nc.sync.dma_start(
                        ach[:], a_d.ap()[mt * P : (mt + 1) * P, c * ACH : (c + 1) * ACH]
                    )
                    ach8 = astreamp.tile([P, ACH], F8, tag="ach8", bufs=2)
                    nc.scalar.activation(
                        ach8[:], ach[:], ACT.Copy, accum_out=dcols[:, c : c + 1]
                    )
                    for g in range(GPC):
                        gk = c * GPC + g  # k-group index 0..15
                        trp = psA.tile([P, 4 * P], F32, tag="trp")
                        for q in range(4):
                            nc.tensor.matmul(
                                trp[:, q * P : (q + 1) * P],
                                ach8[:, (g * 4 + q) * P : (g * 4 + q + 1) * P],
                                identity_f8[:],
                            )
                        dst = at_res[
                            :, (mt * KC + gk * 4) * P : (mt * KC + gk * 4 + 4) * P
                        ]
                        nc.vector.tensor_copy(dst, trp[:])
                nc.vector.reduce_sum(degree_sb[:, mt : mt + 1], dcols[:], axis=AX)

            # ---- AllGather degree ----
            degT_ps = psA.tile([MT, P], F32, tag="trp")
            nc.tensor.transpose(degT_ps[:], degree_sb[:], identity[:])
            degT_sb = smallp.tile([MT, P], F32, tag="degT")
            nc.vector.tensor_copy(degT_sb[:], degT_ps[:])
            nc.sync.dma_start(cin[:], degT_sb[:])
            nc.gpsimd.collective_compute(
                "AllGather",
                ALU.bypass,
                ins=[cin.opt()],
                outs=[cout.opt()],
                replica_groups=[list(range(NCORES))],
            )
            # cout row g = global degree of rows [g*128, (g+1)*128)
            degall_sb = smallp.tile([KC, P], F32, tag="degall")
            nc.sync.dma_start(degall_sb[:], cout[:])
            degallT_ps = psA.tile([P, KC], F32, tag="trp")
            nc.tensor.transpose(degallT_ps[:], degall_sb[:], identity[:KC, :KC])
            recip = degp.tile([P, KC], F32)
            nc.vector.reciprocal(recip[:], degallT_ps[:])
            # ds64_sb[p, g] = 64 / sqrt(degree[g*128 + p])
            ds64_sb = degp.tile([P, KC], F32)
            nc.scalar.activation(ds64_sb[:], recip[:], ACT.Sqrt, scale=4096.0)
            recip8 = degp.tile([P, MT], F32)
            nc.vector.reciprocal(recip8[:], degree_sb[:])
            # dsown64[p, mt] = 1 / (64 * sqrt(degree_own[mt*128 + p]))
            dsown64 = degp.tile([P, MT], F32)
            nc.scalar.activation(dsown64[:], recip8[:], ACT.Sqrt, scale=1.0 / 4096.0)

            # ---- f: stream + park as raw fp8 (no ds dependency -> the f DMA
            # runs right through the collective window), then X'' = ds64 * f8
            f8raw = f8pool.tile([P, KC * D], F8)
            xp_sb = xpp.tile([P, KC * D], F8)  # chunk kc at [:, kc*D:(kc+1)*D]
            f_blk = f_d.ap().rearrange("(b c p) d -> b p c d", c=FCH, p=P)
            for fb in range(KC // FCH):
                fch = fstreamp.tile([P, FCH * D], F32, tag="fch")
                nc.sync.dma_start(
                    fch[:].rearrange("p (c d) -> p c d", c=FCH), f_blk[fb]
                )
                for j in range(FCH):
                    kc = FCH * fb + j
                    nc.scalar.activation(
                        f8raw[:, kc * D : (kc + 1) * D],
                        fch[:, j * D : (j + 1) * D],
                        ACT.Copy,
                    )
            for kc in range(KC):
                nc.vector.tensor_scalar_mul(
                    xp_sb[:, kc * D : (kc + 1) * D],
                    f8raw[:, kc * D : (kc + 1) * D],
                    ds64_sb[:, kc : kc + 1],
                )

            # ---- main matmul: DoubleRow fp8, kc-pair outer ----
            def mm_pair(y_ap, mt, jp, start, stop):
                base = (mt * KC + 2 * jp) * P
                at2 = at_res[:, base : base + 2 * P].rearrange(
                    "p (k m) -> p k m", k=2
                )
                xp2 = xp_sb[:, (2 * jp) * D : (2 * jp + 2) * D].rearrange(
                    "p (k n) -> p k n", k=2
                )
                nc.tensor.matmul(
                    y_ap, at2, xp2, start=start, stop=stop, perf_mode=DR
                )

            ys = [
                psY.tile([P, D], F32, tag="y", name=f"y{i}") for i in range(MTG)
            ]
            for jp in range(NPAIR):
                for mi in range(MTG):
                    mm_pair(ys[mi][:], mi, jp, jp == 0, jp == NPAIR - 1)
            # group 2: m-tiles 6,7 rotate into freed y slots
            ys2 = [
                psY.tile([P, D], F32, tag="y", name=f"y{MTG + i}")
                for i in range(MT - MTG)
            ]
            # epilogue part 1 for group 1: free the Y banks early
            mfs = []
            for mt in range(MTG):
                res = epip.tile([P, D], F32, tag="res")
                nc.sync.dma_start(res[:], fres_d.ap()[mt * P : (mt + 1) * P, :])
                mf = mfpool.tile([P, D], BF16, tag="mf", name=f"mf{mt}")
                nc.vector.scalar_tensor_tensor(
                    mf[:],
                    ys[mt][:],
                    dsown64[:, mt : mt + 1],
                    res[:],
                    op0=ALU.mult,
                    op1=ALU.add,
                )
                mfs.append(mf)
            for jp in range(NPAIR):
                for i, mt in enumerate(range(MTG, MT)):
                    mm_pair(ys2[i][:], mt, jp, jp == 0, jp == NPAIR - 1)
            for i, mt in enumerate(range(MTG, MT)):
                res = epip.tile([P, D], F32, tag="res")
                nc.sync.dma_start(res[:], fres_d.ap()[mt * P : (mt + 1) * P, :])
                mf = mfpool.tile([P, D], BF16, tag="mf", name=f"mf{mt}")
                nc.vector.scalar_tensor_tensor(
                    mf[:],
                    ys2[i][:],
                    dsown64[:, mt : mt + 1],
                    res[:],
                    op0=ALU.mult,
                    op1=ALU.add,
                )
                mfs.append(mf)

            # epilogue part 2: out = relu(mf @ W + b), o accumulators rotate
            # through the freed psY slots
            for mt in range(MT):
                o_ps = psY.tile([P, D], F32, tag="y", name=f"o{mt}")
                for wc in range(4):
                    mfT_ps = psA.tile([P, P], F32, tag="trp")
                    nc.tensor.matmul(
                        mfT_ps[:], mfs[mt][:, wc * P : (wc + 1) * P], identity_bf[:]
                    )
                    mfT_sb = mftp.tile([P, P], BF16, tag="mfT")
                    nc.vector.tensor_copy(mfT_sb[:], mfT_ps[:])
                    nc.tensor.matmul(
                        o_ps[:],
                        mfT_sb[:],
                        w_sb[:, wc * D : (wc + 1) * D],
                        start=(wc == 0),
                        stop=False,
                    )
                nc.tensor.matmul(
                    o_ps[:], ones_row[:], b_sb[:], start=False, stop=True
                )
                osb = epip.tile([P, D], F32, tag="osb")
                nc.scalar.activation(osb[:], o_ps[:], ACT.Relu)
                nc.sync.dma_start(out_d.ap()[mt * P : (mt + 1) * P, :], osb[:])

    nc.compile()
    return nc


def _get_nc():
    if "nc" not in _NC_CACHE:
        _NC_CACHE["nc"] = _build()
    return _NC_CACHE["nc"]


def run(inputs, trace=False, trace_kwargs=None):
    """Run the SPMD kernel; returns (full_output, BassKernelResults)."""
    a = np.ascontiguousarray(np.asarray(inputs["adjacency_matrix"], dtype=np.float32))
    f = np.ascontiguousarray(np.asarray(inputs["feature"], dtype=np.float32))
    w = np.ascontiguousarray(np.asarray(inputs["W"], dtype=np.float32))
    b = np.ascontiguousarray(np.asarray(inputs["b"], dtype=np.float32)).reshape(1, D)

    nc = _get_nc()
    in_maps = []
    for d in range(NCORES):
        rows = slice(d * R, (d + 1) * R)
        in_maps.append({"a": a[rows], "f": f, "fres": f[rows], "w": w, "bias": b})
    res = bass_utils.run_bass_kernel_spmd(
        nc,
        in_maps,
        core_ids=list(range(NCORES)),
        trace=trace,
        **(trace_kwargs or {}),
    )
    out = np.concatenate([r["out"] for r in res.results], axis=0)
    return out, res


def kernel(**inputs):
    out, _ = run(inputs, trace=False)
    return out


# revision 6
# speedup vs baseline: 1.5834x; 1.0433x over previous
"""GNN message-passing layer (LplsNorm + residual conv) on 8 Trainium2 cores.

Computation (reference, all f32):
    degree = A.sum(-1); ds = degree**-0.5
    mf  = f + ds[:,None] * (A @ (ds[:,None] * f))      # a_norm = ds A ds
    out = relu(mf @ W + b)
Distribution: A row-sharded over 8 cores ([1024, 8192] each), feature
replicated.

v6 schedule (fp8 DoubleRow + split degree AllGather):
  - Phase 1: stream the A shard once (DMA-bound ~330 GB/s; f loads are
    dependency-pinned behind the A stream so they cannot steal phase-1
    bandwidth). ScalarE casts chunks to fp8-e4m3 while accumulating exact
    f32 row sums; TensorE transposes fp8 tiles; DVE copies them into an
    SBUF-resident transposed-A store (8 MiB fp8, no spill).
  - A mid-kernel 4 KB AllGather costs ~50 us wall (mesh firmware latency),
    so degrees ship in TWO collectives: m-tiles 0-3 at ~58% of the stream
    (lands before the stream ends) and m-tiles 4-7 at the end. The second
    collective's window is bridged by real matmuls on the first half's
    k-chunks, with f streaming/parking (raw fp8, no ds needed) running
    underneath; paced dummy transposes keep the PE's HAM clock warm.
  - The A@X term contributes only ~0.7% of mf (the residual dominates), so
    fp8 keeps total l2 rel err ~2e-3 (verified vs numpy; gate 2e-2).
  - Main matmul: DoubleRow fp8 (2 k-chunks/instruction). M-tiles 0-5
    accumulate in 6 PSUM banks; m-tiles 6-7 + epilogue o-accumulators
    rotate through the same pool as banks free up. psA's 2 banks serve
    phase-1 transposes, then the epilogue's mf transposes.
  - Epilogue: mf = Y * dsown/64 + f_res (DVE, bf16), mf @ W in bf16, bias
    via a K=1 bf16 matmul, ACT relu, store.
"""

import numpy as np

import concourse.bass as bass
import concourse.mybir as mybir
import concourse.tile as tile
from concourse import bacc
from concourse import bass_utils
from concourse.masks import make_identity

N = 8192
D = 512
NCORES = 8
P = 128
R = N // NCORES          # rows per core: 1024
MT = R // P              # m-tiles per core: 8
KC = N // P              # k-chunks: 64
ACH = 2048               # A stream chunk width (f32 -> 1 MiB per DMA)
NACH = N // ACH          # stream chunks per row-block: 4
GPC = ACH // (4 * P)     # transpose groups (of 4 tiles) per stream chunk: 4
MTG = 6                  # m-tiles in the big matmul group (PSUM banks)
HMT = 4                  # m-tiles per degree-collective half

F32 = mybir.dt.float32
BF16 = mybir.dt.bfloat16
F8 = mybir.dt.float8e4

_NC_CACHE = {}


def _build():
    nc = bacc.Bacc("TRN2", target_bir_lowering=False, debug=False, num_devices=NCORES)

    a_d = nc.dram_tensor("a", [R, N], F32, kind="ExternalInput")
    f_d = nc.dram_tensor("f", [N, D], F32, kind="ExternalInput")
    fres_d = nc.dram_tensor("fres", [R, D], F32, kind="ExternalInput")
    w_d = nc.dram_tensor("w", [D, D], F32, kind="ExternalInput")
    b_d = nc.dram_tensor("bias", [1, D], F32, kind="ExternalInput")
    out_d = nc.dram_tensor("out", [R, D], F32, kind="ExternalOutput")

    AX = mybir.AxisListType.X
    ALU = mybir.AluOpType
    ACT = mybir.ActivationFunctionType
    DR = mybir.MatmulPerfMode.DoubleRow

    with tile.TileContext(nc) as tc:
        with (
            tc.tile_pool(name="const", bufs=1) as constp,
            tc.tile_pool(name="deg", bufs=1) as degp,
            tc.tile_pool(name="astream", bufs=2) as astreamp,
            tc.tile_pool(name="small", bufs=2) as smallp,
            tc.tile_pool(name="atres", bufs=1) as atresp,
            tc.tile_pool(name="xp", bufs=1) as xpp,
            tc.tile_pool(name="f8p", bufs=1) as f8pool,
            tc.tile_pool(name="fstream", bufs=3) as fstreamp,
            tc.tile_pool(name="epi", bufs=2) as epip,
            tc.tile_pool(name="mfp", bufs=MT) as mfpool,
            tc.tile_pool(name="mft", bufs=2) as mftp,
            tc.tile_pool(name="psA", bufs=2, space="PSUM") as psA,      # transposes
            tc.tile_pool(name="psY", bufs=MTG, space="PSUM") as psY,    # Y + o accum
            tc.tile_pool(name="dram", bufs=1, space="DRAM") as dramp,
        ):
            # ---- constants ----
            identity = constp.tile([P, P], F32)
            make_identity(nc, identity[:])
            identity_f8 = constp.tile([P, P], F8)
            make_identity(nc, identity_f8[:])
            identity_bf = constp.tile([P, P], BF16)
            make_identity(nc, identity_bf[:])
            ones_row = constp.tile([1, P], BF16)
            nc.gpsimd.memset(ones_row[:], 1.0)
            bf_sb = constp.tile([1, D], F32)
            nc.sync.dma_start(bf_sb[:], b_d.ap())
            b_sb = constp.tile([1, D], BF16)
            nc.vector.tensor_copy(b_sb[:], bf_sb[:])
            wf_sb = constp.tile([P, 4 * D], F32)  # w chunk wc at [:, wc*D:(wc+1)*D]
            for wc in range(4):
                nc.sync.dma_start(
                    wf_sb[:, wc * D : (wc + 1) * D], w_d.ap()[wc * P : (wc + 1) * P, :]
                )
            w_sb = constp.tile([P, 4 * D], BF16)
            nc.vector.tensor_copy(w_sb[:], wf_sb[:])

            # resident transposed-A store: (mt, kc) tile at col (mt*KC+kc)*P
            at_res = atresp.tile([P, MT * KC * P], F8)
            cin = [dramp.tile([HMT, P], F32, name=f"cin{h}") for h in range(2)]
            cout = [
                dramp.tile([NCORES * HMT, P], F32, name=f"cout{h}") for h in range(2)
            ]

            degree_sb = degp.tile([P, MT], F32)  # col mt = degree of rows mt*128..
            # ds64_sb[p, e*8 + mt] = 64 / sqrt(degree[global row e*1024+mt*128+p])
            ds64_sb = degp.tile([P, KC], F32)
            ds64_v = ds64_sb[:].rearrange("p (e c) -> p e c", e=NCORES)

            def issue_degree_half(h):
                """Transpose degree cols [h*4, h*4+4), AllGather them."""
                degTh_ps = psA.tile([HMT, P], F32, tag="trp", name=f"degT{h}")
                nc.tensor.matmul(
                    degTh_ps[:],
                    degree_sb[:, h * HMT : (h + 1) * HMT],
                    identity[:],
                )
                degTh_sb = smallp.tile([HMT, P], F32, tag="degT", name=f"degTs{h}")
                nc.vector.tensor_copy(degTh_sb[:], degTh_ps[:])
                nc.sync.dma_start(cin[h][:], degTh_sb[:])
                nc.gpsimd.collective_compute(
                    "AllGather",
                    ALU.bypass,
                    ins=[cin[h].opt()],
                    outs=[cout[h].opt()],
                    replica_groups=[list(range(NCORES))],
                )

            def consume_degree_half(h):
                """cout[h] row e*4+t = degree of rows [e*1024+(h*4+t)*128, +128)."""
                degall = smallp.tile(
                    [NCORES * HMT, P], F32, tag="degall", name=f"dga{h}"
                )
                nc.sync.dma_start(degall[:], cout[h][:])
                degallT_ps = psA.tile(
                    [P, NCORES * HMT], F32, tag="trp", name=f"dgaT{h}"
                )
                nc.tensor.transpose(
                    degallT_ps[:], degall[:], identity[: NCORES * HMT, : NCORES * HMT]
                )
                reciph = degp.tile([P, NCORES * HMT], F32, name=f"reciph{h}")
                nc.vector.reciprocal(reciph[:], degallT_ps[:])
                nc.scalar.activation(
                    ds64_v[:, :, h * HMT : (h + 1) * HMT],
                    reciph[:].rearrange("p (e c) -> p e c", e=NCORES),
                    ACT.Sqrt,
                    scale=4096.0,
                )

            # ---- merged pass: degree + transpose-all; half-collectives ----
            for mt in range(MT):
                dcols = smallp.tile([P, NACH], F32, tag="dcols")
                for c in range(NACH):
                    ach = astreamp.tile([P, ACH], F32, tag="ach")
                    nc.sync.dma_start(
                        ach[:], a_d.ap()[mt * P : (mt + 1) * P, c * ACH : (c + 1) * ACH]
                    )
                    ach8 = astreamp.tile([P, ACH], F8, tag="ach8", bufs=2)
                    nc.scalar.activation(
                        ach8[:], ach[:], ACT.Copy, accum_out=dcols[:, c : c + 1]
                    )
                    for g in range(GPC):
                        gk = c * GPC + g  # k-group index 0..15
                        trp = psA.tile([P, 4 * P], F32, tag="trp")
                        for q in range(4):
                            nc.tensor.matmul(
                                trp[:, q * P : (q + 1) * P],
                                ach8[:, (g * 4 + q) * P : (g * 4 + q + 1) * P],
                                identity_f8[:],
                            )
                        dst = at_res[
                            :, (mt * KC + gk * 4) * P : (mt * KC + gk * 4 + 4) * P
                        ]
                        nc.vector.tensor_copy(dst, trp[:])
                nc.vector.reduce_sum(degree_sb[:, mt : mt + 1], dcols[:], axis=AX)
                if mt == HMT - 1:
                    issue_degree_half(0)
            issue_degree_half(1)

            recip8 = degp.tile([P, MT], F32)
            nc.vector.reciprocal(recip8[:], degree_sb[:])
            # dsown64[p, mt] = 1 / (64 * sqrt(degree_own[mt*128 + p]))
            dsown64 = degp.tile([P, MT], F32)
            nc.scalar.activation(dsown64[:], recip8[:], ACT.Sqrt, scale=1.0 / 4096.0)

            # ---- f: stream + park as raw fp8, half h (= k-chunks e*8+4h..+3)
            # first. The first fch buffers take a fake WAW dep on degree_sb so
            # the f stream cannot start before the A stream is done.
            f8raw = f8pool.tile([P, KC * D], F8)
            xp_sb = xpp.tile([P, KC * D], F8)  # chunk kc at [:, kc*D:(kc+1)*D]
            f_blk = f_d.ap().rearrange(
                "(e h c p) d -> h e p c d", h=2, c=HMT, p=P
            )
            nfch = 0
            for h in range(2):
                for e in range(NCORES):
                    fch = fstreamp.tile([P, HMT * D], F32, tag="fch")
                    if nfch < 3:
                        nc.vector.tensor_copy(fch[:, :1], degree_sb[:, MT - 1 : MT])
                    nfch += 1
                    nc.sync.dma_start(
                        fch[:].rearrange("p (c d) -> p c d", c=HMT), f_blk[h, e]
                    )
                    for j in range(HMT):
                        kc = e * 8 + h * HMT + j
                        nc.scalar.activation(
                            f8raw[:, kc * D : (kc + 1) * D],
                            fch[:, j * D : (j + 1) * D],
                            ACT.Copy,
                        )

            def make_xpp(h):
                consume_degree_half(h)
                for e in range(NCORES):
                    for j in range(HMT):
                        kc = e * 8 + h * HMT + j
                        nc.vector.tensor_scalar_mul(
                            xp_sb[:, kc * D : (kc + 1) * D],
                            f8raw[:, kc * D : (kc + 1) * D],
                            ds64_sb[:, kc : kc + 1],
                        )

            # ---- main matmul: DoubleRow fp8 over (e, half, pair) ----
            def mm_pair(y_ap, mt, pj, start, stop):
                """pj = global k-pair index (k-chunks 2*pj, 2*pj+1)."""
                base = (mt * KC + 2 * pj) * P
                at2 = at_res[:, base : base + 2 * P].rearrange(
                    "p (k m) -> p k m", k=2
                )
                xp2 = xp_sb[:, (2 * pj) * D : (2 * pj + 2) * D].rearrange(
                    "p (k n) -> p k n", k=2
                )
                nc.tensor.matmul(
                    y_ap, at2, xp2, start=start, stop=stop, perf_mode=DR
                )

            def half_pairs(h):
                """k-pairs of half h in park order: (e, t) -> pair 4e+2h+t."""
                return [
                    (e, 4 * e + 2 * h + t) for e in range(NCORES) for t in range(2)
                ]

            ys = [
                psY.tile([P, D], F32, tag="y", name=f"y{i}") for i in range(MTG)
            ]
            make_xpp(0)
            for e, pj in half_pairs(0):
                for mi in range(MTG):
                    mm_pair(ys[mi][:], mi, pj, pj == 2 * 0, False)
            # dummy transposes paced by the second-half f parks keep HAM warm
            # across the second collective's window
            for e in range(NCORES):
                for t in range(2):
                    kc = e * 8 + 4 + 2 * t
                    dps = psA.tile([P, P], F32, tag="trp", name=f"dum{e}_{t}")
                    nc.tensor.matmul(
                        dps[:], identity_f8[:], f8raw[:, kc * D : kc * D + P]
                    )
            make_xpp(1)
            for e, pj in half_pairs(1):
                for mi in range(MTG):
                    mm_pair(ys[mi][:], mi, pj, False, pj == 4 * (NCORES - 1) + 3)

            # group 2: m-tiles 6,7 rotate into freed y slots
            ys2 = [
                psY.tile([P, D], F32, tag="y", name=f"y{MTG + i}")
                for i in range(MT - MTG)
            ]
            # epilogue part 1 for group 1: free the Y banks early
            mfs = []
            for mt in range(MTG):
                res = epip.tile([P, D], F32, tag="res")
                nc.sync.dma_start(res[:], fres_d.ap()[mt * P : (mt + 1) * P, :])
                mf = mfpool.tile([P, D], BF16, tag="mf", name=f"mf{mt}")
                nc.vector.scalar_tensor_tensor(
                    mf[:],
                    ys[mt][:],
                    dsown64[:, mt : mt + 1],
                    res[:],
                    op0=ALU.mult,
                    op1=ALU.add,
                )
                mfs.append(mf)
            for h in range(2):
                for e, pj in half_pairs(h):
                    for i, mt in enumerate(range(MTG, MT)):
                        mm_pair(
                            ys2[i][:],
                            mt,
                            pj,
                            pj == 0,
                            pj == 4 * (NCORES - 1) + 3,
                        )
            for i, mt in enumerate(range(MTG, MT)):
                res = epip.tile([P, D], F32, tag="res")
                nc.sync.dma_start(res[:], fres_d.ap()[mt * P : (mt + 1) * P, :])
                mf = mfpool.tile([P, D], BF16, tag="mf", name=f"mf{mt}")
                nc.vector.scalar_tensor_tensor(
                    mf[:],
                    ys2[i][:],
                    dsown64[:, mt : mt + 1],
                    res[:],
                    op0=ALU.mult,
                    op1=ALU.add,
                )
                mfs.append(mf)

            # epilogue part 2: out = relu(mf @ W + b), o accumulators rotate
            # through the freed psY slots
            for mt in range(MT):
                o_ps = psY.tile([P, D], F32, tag="y", name=f"o{mt}")
                for wc in range(4):
                    mfT_ps = psA.tile([P, P], F32, tag="trp")
                    nc.tensor.matmul(
                        mfT_ps[:], mfs[mt][:, wc * P : (wc + 1) * P], identity_bf[:]
                    )
                    mfT_sb = mftp.tile([P, P], BF16, tag="mfT")
                    nc.vector.tensor_copy(mfT_sb[:], mfT_ps[:])
                    nc.tensor.matmul(
                        o_ps[:],
                        mfT_sb[:],
                        w_sb[:, wc * D : (wc + 1) * D],
                        start=(wc == 0),
                        stop=False,
                    )
                nc.tensor.matmul(
                    o_ps[:], ones_row[:], b_sb[:], start=False, stop=True
                )
                osb = epip.tile([P, D], F32, tag="osb")
                nc.scalar.activation(osb[:], o_ps[:], ACT.Relu)
                nc.sync.dma_start(out_d.ap()[mt * P : (mt + 1) * P, :], osb[:])

    nc.compile()
    return nc


def _get_nc():
    if "nc" not in _NC_CACHE:
        _NC_CACHE["nc"] = _build()
    return _NC_CACHE["nc"]


def run(inputs, trace=False, trace_kwargs=None):
    """Run the SPMD kernel; returns (full_output, BassKernelResults)."""
    a = np.ascontiguousarray(np.asarray(inputs["adjacency_matrix"], dtype=np.float32))
    f = np.ascontiguousarray(np.asarray(inputs["feature"], dtype=np.float32))
    w = np.ascontiguousarray(np.asarray(inputs["W"], dtype=np.float32))
    b = np.ascontiguousarray(np.asarray(inputs["b"], dtype=np.float32)).reshape(1, D)

    nc = _get_nc()
    in_maps = []
    for d in range(NCORES):
        rows = slice(d * R, (d + 1) * R)
        in_maps.append({"a": a[rows], "f": f, "fres": f[rows], "w": w, "bias": b})
    res = bass_utils.run_bass_kernel_spmd(
        nc,
        in_maps,
        core_ids=list(range(NCORES)),
        trace=trace,
        **(trace_kwargs or {}),
    )
    out = np.concatenate([r["out"] for r in res.results], axis=0)
    return out, res


def kernel(**inputs):
    out, _ = run(inputs, trace=False)
    return out


# revision 11
# speedup vs baseline: 1.8359x; 1.1595x over previous
"""GNN message-passing layer (LplsNorm + residual conv) on 8 Trainium2 cores.

Computation (reference, all f32):
    degree = A.sum(-1); ds = degree**-0.5
    mf  = f + ds[:,None] * (A @ (ds[:,None] * f))      # a_norm = ds A ds
    out = relu(mf @ W + b)
Distribution: A row-sharded over 8 cores ([1024, 8192] each), feature
replicated.

v6 schedule (fp8 DoubleRow + split degree AllGather):
  - Phase 1: stream the A shard once (DMA-bound ~330 GB/s; f loads are
    dependency-pinned behind the A stream so they cannot steal phase-1
    bandwidth). ScalarE casts chunks to fp8-e4m3 while accumulating exact
    f32 row sums; TensorE transposes fp8 tiles; DVE copies them into an
    SBUF-resident transposed-A store (8 MiB fp8, no spill).
  - A mid-kernel 4 KB AllGather costs ~50 us wall (mesh firmware latency),
    so degrees ship in TWO collectives: m-tiles 0-3 at ~58% of the stream
    (lands before the stream ends) and m-tiles 4-7 at the end. The second
    collective's window is bridged by real matmuls on the first half's
    k-chunks, with f streaming/parking (raw fp8, no ds needed) running
    underneath; paced dummy transposes keep the PE's HAM clock warm.
  - The A@X term contributes only ~0.7% of mf (the residual dominates), so
    fp8 keeps total l2 rel err ~2e-3 (verified vs numpy; gate 2e-2).
  - Main matmul: DoubleRow fp8 (2 k-chunks/instruction). M-tiles 0-5
    accumulate in 6 PSUM banks; m-tiles 6-7 + epilogue o-accumulators
    rotate through the same pool as banks free up. psA's 2 banks serve
    phase-1 transposes, then the epilogue's mf transposes.
  - Epilogue: mf = Y * dsown/64 + f_res (DVE, bf16), mf @ W in bf16, bias
    via a K=1 bf16 matmul, ACT relu, store.
"""

import numpy as np

import concourse.bass as bass
import concourse.mybir as mybir
import concourse.tile as tile
from concourse import bacc
from concourse import bass_utils
from concourse.masks import make_identity

N = 8192
D = 512
NCORES = 8
P = 128
R = N // NCORES          # rows per core: 1024
MT = R // P              # m-tiles per core: 8
KC = N // P              # k-chunks: 64
ACH = 2048               # A stream chunk width (f32 -> 1 MiB per DMA)
NACH = N // ACH          # stream chunks per row-block: 4
GPC = ACH // (4 * P)     # transpose groups (of 4 tiles) per stream chunk: 4
MTG = 6                  # m-tiles in the big matmul group (PSUM banks)
HMT = 4                  # m-tiles per degree-collective half

F32 = mybir.dt.float32
BF16 = mybir.dt.bfloat16
F8 = mybir.dt.float8e4

_NC_CACHE = {}


def _build():
    nc = bacc.Bacc("TRN2", target_bir_lowering=False, debug=False, num_devices=NCORES)

    a_d = nc.dram_tensor("a", [R, N], F32, kind="ExternalInput")
    f_d = nc.dram_tensor("f", [N, D], F32, kind="ExternalInput")
    fres_d = nc.dram_tensor("fres", [R, D], F32, kind="ExternalInput")
    w_d = nc.dram_tensor("w", [D, D], F32, kind="ExternalInput")
    b_d = nc.dram_tensor("bias", [1, D], F32, kind="ExternalInput")
    out_d = nc.dram_tensor("out", [R, D], F32, kind="ExternalOutput")

    AX = mybir.AxisListType.X
    ALU = mybir.AluOpType
    ACT = mybir.ActivationFunctionType
    DR = mybir.MatmulPerfMode.DoubleRow

    with tile.TileContext(nc) as tc:
        with (
            tc.tile_pool(name="const", bufs=1) as constp,
            tc.tile_pool(name="deg", bufs=1) as degp,
            tc.tile_pool(name="astream", bufs=3) as astreamp,
            tc.tile_pool(name="small", bufs=2) as smallp,
            tc.tile_pool(name="atres", bufs=1) as atresp,
            tc.tile_pool(name="xp", bufs=1) as xpp,
            tc.tile_pool(name="f8p", bufs=1) as f8pool,
            tc.tile_pool(name="fstream", bufs=2) as fstreamp,
            tc.tile_pool(name="epi", bufs=2) as epip,
            tc.tile_pool(name="mfp", bufs=MT) as mfpool,
            tc.tile_pool(name="mft", bufs=2) as mftp,
            tc.tile_pool(name="psA", bufs=2, space="PSUM") as psA,      # transposes
            tc.tile_pool(name="psY", bufs=MTG, space="PSUM") as psY,    # Y + o accum
            tc.tile_pool(name="dram", bufs=1, space="DRAM") as dramp,
        ):
            # ---- constants ----
            identity = constp.tile([P, P], F32)
            make_identity(nc, identity[:])
            identity_f8 = constp.tile([P, P], F8)
            make_identity(nc, identity_f8[:])
            identity_bf = constp.tile([P, P], BF16)
            make_identity(nc, identity_bf[:])
            ones_row = constp.tile([1, P], BF16)
            nc.gpsimd.memset(ones_row[:], 1.0)
            bf_sb = constp.tile([1, D], F32)
            nc.sync.dma_start(bf_sb[:], b_d.ap())
            b_sb = constp.tile([1, D], BF16)
            nc.vector.tensor_copy(b_sb[:], bf_sb[:])
            w_sb = constp.tile([P, 4 * D], BF16)  # w chunk wc at [:, wc*D:(wc+1)*D]
            for wc in range(4):
                wf_ch = smallp.tile([P, D], F32, tag="wf")
                nc.sync.dma_start(wf_ch[:], w_d.ap()[wc * P : (wc + 1) * P, :])
                nc.vector.tensor_copy(w_sb[:, wc * D : (wc + 1) * D], wf_ch[:])

            # resident transposed-A store: (mt, kc) tile at col (mt*KC+kc)*P
            at_res = atresp.tile([P, MT * KC * P], F8)
            cin = [dramp.tile([HMT, P], F32, name=f"cin{h}") for h in range(2)]
            cout = [
                dramp.tile([NCORES * HMT, P], F32, name=f"cout{h}") for h in range(2)
            ]

            degree_sb = degp.tile([P, MT], F32)  # col mt = degree of rows mt*128..
            # ds64_sb[p, e*8 + mt] = 64 / sqrt(degree[global row e*1024+mt*128+p])
            ds64_sb = degp.tile([P, KC], F32)
            ds64_v = ds64_sb[:].rearrange("p (e c) -> p e c", e=NCORES)

            def issue_degree_half(h):
                """Transpose degree cols [h*4, h*4+4), AllGather them."""
                degTh_ps = psA.tile([HMT, P], F32, tag="trp", name=f"degT{h}")
                nc.tensor.matmul(
                    degTh_ps[:],
                    degree_sb[:, h * HMT : (h + 1) * HMT],
                    identity[:],
                )
                degTh_sb = smallp.tile([HMT, P], F32, tag="degT", name=f"degTs{h}")
                nc.vector.tensor_copy(degTh_sb[:], degTh_ps[:])
                nc.sync.dma_start(cin[h][:], degTh_sb[:])
                nc.gpsimd.collective_compute(
                    "AllGather",
                    ALU.bypass,
                    ins=[cin[h].opt()],
                    outs=[cout[h].opt()],
                    replica_groups=[list(range(NCORES))],
                )

            def consume_degree_half(h):
                """cout[h] row e*4+t = degree of rows [e*1024+(h*4+t)*128, +128)."""
                degall = smallp.tile(
                    [NCORES * HMT, P], F32, tag="degall", name=f"dga{h}"
                )
                nc.sync.dma_start(degall[:], cout[h][:])
                degallT_ps = psA.tile(
                    [P, NCORES * HMT], F32, tag="trp", name=f"dgaT{h}"
                )
                nc.tensor.transpose(
                    degallT_ps[:], degall[:], identity[: NCORES * HMT, : NCORES * HMT]
                )
                reciph = degp.tile([P, NCORES * HMT], F32, name=f"reciph{h}")
                nc.vector.reciprocal(reciph[:], degallT_ps[:])
                nc.scalar.activation(
                    ds64_v[:, :, h * HMT : (h + 1) * HMT],
                    reciph[:].rearrange("p (e c) -> p e c", e=NCORES),
                    ACT.Sqrt,
                    scale=4096.0,
                )

            # ---- merged pass: degree + transpose-all; half-collectives ----
            for mt in range(MT):
                dcols = smallp.tile([P, NACH], F32, tag="dcols")
                for c in range(NACH):
                    ach = astreamp.tile([P, ACH], F32, tag="ach")
                    nc.sync.dma_start(
                        ach[:], a_d.ap()[mt * P : (mt + 1) * P, c * ACH : (c + 1) * ACH]
                    )
                    ach8 = astreamp.tile([P, ACH], F8, tag="ach8", bufs=2)
                    nc.scalar.activation(
                        ach8[:], ach[:], ACT.Copy, accum_out=dcols[:, c : c + 1]
                    )
                    for g in range(GPC):
                        gk = c * GPC + g  # k-group index 0..15
                        trp = psA.tile([P, 4 * P], F32, tag="trp")
                        for q in range(4):
                            nc.tensor.matmul(
                                trp[:, q * P : (q + 1) * P],
                                ach8[:, (g * 4 + q) * P : (g * 4 + q + 1) * P],
                                identity_f8[:],
                            )
                        dst = at_res[
                            :, (mt * KC + gk * 4) * P : (mt * KC + gk * 4 + 4) * P
                        ]
                        nc.vector.tensor_copy(dst, trp[:])
                nc.vector.reduce_sum(degree_sb[:, mt : mt + 1], dcols[:], axis=AX)
                if mt == HMT - 1:
                    issue_degree_half(0)
            issue_degree_half(1)

            recip8 = degp.tile([P, MT], F32)
            nc.vector.reciprocal(recip8[:], degree_sb[:])
            # dsown64[p, mt] = 1 / (64 * sqrt(degree_own[mt*128 + p]))
            dsown64 = degp.tile([P, MT], F32)
            nc.scalar.activation(dsown64[:], recip8[:], ACT.Sqrt, scale=1.0 / 4096.0)

            # ---- f: stream + park as raw fp8, half h (= k-chunks e*8+4h..+3)
            # first. The first fch buffers take a fake WAW dep on degree_sb so
            # the f stream cannot start before the A stream is done.
            f8raw = f8pool.tile([P, KC * D], F8)
            xp_sb = xpp.tile([P, KC * D], F8)  # chunk kc at [:, kc*D:(kc+1)*D]
            f_blk = f_d.ap().rearrange(
                "(e h c p) d -> h e p c d", h=2, c=HMT, p=P
            )
            nfch = 0
            for h in range(2):
                for e in range(NCORES):
                    fch = fstreamp.tile([P, HMT * D], F32, tag="fch")
                    if nfch < 3:
                        nc.vector.tensor_copy(fch[:, :1], degree_sb[:, MT - 1 : MT])
                    nfch += 1
                    nc.sync.dma_start(
                        fch[:].rearrange("p (c d) -> p c d", c=HMT), f_blk[h, e]
                    )
                    for j in range(HMT):
                        kc = e * 8 + h * HMT + j
                        # split the f32->fp8 park across ScalarE and DVE so it
                        # keeps pace with the f DMA stream
                        if j % 2 == 0:
                            nc.scalar.activation(
                                f8raw[:, kc * D : (kc + 1) * D],
                                fch[:, j * D : (j + 1) * D],
                                ACT.Copy,
                            )
                        else:
                            nc.vector.tensor_copy(
                                f8raw[:, kc * D : (kc + 1) * D],
                                fch[:, j * D : (j + 1) * D],
                            )

            def make_xpp(h):
                consume_degree_half(h)
                for e in range(NCORES):
                    for j in range(HMT):
                        kc = e * 8 + h * HMT + j
                        nc.vector.tensor_scalar_mul(
                            xp_sb[:, kc * D : (kc + 1) * D],
                            f8raw[:, kc * D : (kc + 1) * D],
                            ds64_sb[:, kc : kc + 1],
                        )

            # ---- main matmul: DoubleRow fp8 over (e, half, pair) ----
            def mm_pair(y_ap, mt, pj, start, stop):
                """pj = global k-pair index (k-chunks 2*pj, 2*pj+1)."""
                base = (mt * KC + 2 * pj) * P
                at2 = at_res[:, base : base + 2 * P].rearrange(
                    "p (k m) -> p k m", k=2
                )
                xp2 = xp_sb[:, (2 * pj) * D : (2 * pj + 2) * D].rearrange(
                    "p (k n) -> p k n", k=2
                )
                nc.tensor.matmul(
                    y_ap, at2, xp2, start=start, stop=stop, perf_mode=DR
                )

            def half_pairs(h):
                """k-pairs of half h in park order: (e, t) -> pair 4e+2h+t."""
                return [
                    (e, 4 * e + 2 * h + t) for e in range(NCORES) for t in range(2)
                ]

            ys = [
                psY.tile([P, D], F32, tag="y", name=f"y{i}") for i in range(MTG)
            ]
            make_xpp(0)
            for e, pj in half_pairs(0):
                for mi in range(MTG):
                    mm_pair(ys[mi][:], mi, pj, pj == 2 * 0, False)
            make_xpp(1)
            for e, pj in half_pairs(1):
                for mi in range(MTG):
                    mm_pair(ys[mi][:], mi, pj, False, pj == 4 * (NCORES - 1) + 3)

            # group 2: m-tiles 6,7 rotate into freed y slots
            ys2 = [
                psY.tile([P, D], F32, tag="y", name=f"y{MTG + i}")
                for i in range(MT - MTG)
            ]
            # epilogue part 1 for group 1: free the Y banks early
            mfs = []
            for mt in range(MTG):
                res = epip.tile([P, D], F32, tag="res")
                nc.sync.dma_start(res[:], fres_d.ap()[mt * P : (mt + 1) * P, :])
                mf = mfpool.tile([P, D], BF16, tag="mf", name=f"mf{mt}")
                nc.vector.scalar_tensor_tensor(
                    mf[:],
                    ys[mt][:],
                    dsown64[:, mt : mt + 1],
                    res[:],
                    op0=ALU.mult,
                    op1=ALU.add,
                )
                mfs.append(mf)
            for h in range(2):
                for e, pj in half_pairs(h):
                    for i, mt in enumerate(range(MTG, MT)):
                        mm_pair(
                            ys2[i][:],
                            mt,
                            pj,
                            pj == 0,
                            pj == 4 * (NCORES - 1) + 3,
                        )
            for i, mt in enumerate(range(MTG, MT)):
                res = epip.tile([P, D], F32, tag="res")
                nc.sync.dma_start(res[:], fres_d.ap()[mt * P : (mt + 1) * P, :])
                mf = mfpool.tile([P, D], BF16, tag="mf", name=f"mf{mt}")
                nc.vector.scalar_tensor_tensor(
                    mf[:],
                    ys2[i][:],
                    dsown64[:, mt : mt + 1],
                    res[:],
                    op0=ALU.mult,
                    op1=ALU.add,
                )
                mfs.append(mf)

            # epilogue part 2: out = relu(mf @ W + b), o accumulators rotate
            # through the freed psY slots
            for mt in range(MT):
                o_ps = psY.tile([P, D], F32, tag="y", name=f"o{mt}")
                for wc in range(4):
                    mfT_ps = psA.tile([P, P], F32, tag="trp")
                    nc.tensor.matmul(
                        mfT_ps[:], mfs[mt][:, wc * P : (wc + 1) * P], identity_bf[:]
                    )
                    mfT_sb = mftp.tile([P, P], BF16, tag="mfT")
                    nc.vector.tensor_copy(mfT_sb[:], mfT_ps[:])
                    nc.tensor.matmul(
                        o_ps[:],
                        mfT_sb[:],
                        w_sb[:, wc * D : (wc + 1) * D],
                        start=(wc == 0),
                        stop=False,
                    )
                nc.tensor.matmul(
                    o_ps[:], ones_row[:], b_sb[:], start=False, stop=True
                )
                osb = epip.tile([P, D], F32, tag="osb")
                nc.scalar.activation(osb[:], o_ps[:], ACT.Relu)
                nc.sync.dma_start(out_d.ap()[mt * P : (mt + 1) * P, :], osb[:])

    nc.compile()
    return nc


def _get_nc():
    if "nc" not in _NC_CACHE:
        _NC_CACHE["nc"] = _build()
    return _NC_CACHE["nc"]


def run(inputs, trace=False, trace_kwargs=None):
    """Run the SPMD kernel; returns (full_output, BassKernelResults)."""
    a = np.ascontiguousarray(np.asarray(inputs["adjacency_matrix"], dtype=np.float32))
    f = np.ascontiguousarray(np.asarray(inputs["feature"], dtype=np.float32))
    w = np.ascontiguousarray(np.asarray(inputs["W"], dtype=np.float32))
    b = np.ascontiguousarray(np.asarray(inputs["b"], dtype=np.float32)).reshape(1, D)

    nc = _get_nc()
    in_maps = []
    for d in range(NCORES):
        rows = slice(d * R, (d + 1) * R)
        in_maps.append({"a": a[rows], "f": f, "fres": f[rows], "w": w, "bias": b})
    res = bass_utils.run_bass_kernel_spmd(
        nc,
        in_maps,
        core_ids=list(range(NCORES)),
        trace=trace,
        **(trace_kwargs or {}),
    )
    out = np.concatenate([r["out"] for r in res.results], axis=0)
    return out, res


def kernel(**inputs):
    out, _ = run(inputs, trace=False)
    return out


# revision 14
# speedup vs baseline: 1.9721x; 1.0742x over previous
"""GNN message-passing layer (LplsNorm + residual conv) on 8 Trainium2 cores.

Computation (reference, all f32):
    degree = A.sum(-1); ds = degree**-0.5
    mf  = f + ds[:,None] * (A @ (ds[:,None] * f))      # a_norm = ds A ds
    out = relu(mf @ W + b)
Distribution: A row-sharded over 8 cores ([1024, 8192] each), feature
replicated.

v6 schedule (fp8 DoubleRow + split degree AllGather):
  - Phase 1: stream the A shard once (DMA-bound ~330 GB/s; f loads are
    dependency-pinned behind the A stream so they cannot steal phase-1
    bandwidth). ScalarE casts chunks to fp8-e4m3 while accumulating exact
    f32 row sums; TensorE transposes fp8 tiles; DVE copies them into an
    SBUF-resident transposed-A store (8 MiB fp8, no spill).
  - A mid-kernel 4 KB AllGather costs ~50 us wall (mesh firmware latency),
    so degrees ship in TWO collectives: m-tiles 0-3 at ~58% of the stream
    (lands before the stream ends) and m-tiles 4-7 at the end. The second
    collective's window is bridged by real matmuls on the first half's
    k-chunks, with f streaming/parking (raw fp8, no ds needed) running
    underneath; paced dummy transposes keep the PE's HAM clock warm.
  - The A@X term contributes only ~0.7% of mf (the residual dominates), so
    fp8 keeps total l2 rel err ~2e-3 (verified vs numpy; gate 2e-2).
  - Main matmul: DoubleRow fp8 (2 k-chunks/instruction). M-tiles 0-5
    accumulate in 6 PSUM banks; m-tiles 6-7 + epilogue o-accumulators
    rotate through the same pool as banks free up. psA's 2 banks serve
    phase-1 transposes, then the epilogue's mf transposes.
  - Epilogue: mf = Y * dsown/64 + f_res (DVE, bf16), mf @ W in bf16, bias
    via a K=1 bf16 matmul, ACT relu, store.
"""

import numpy as np

import concourse.bass as bass
import concourse.mybir as mybir
import concourse.tile as tile
from concourse import bacc
from concourse import bass_utils
from concourse.masks import make_identity

N = 8192
D = 512
NCORES = 8
P = 128
R = N // NCORES          # rows per core: 1024
MT = R // P              # m-tiles per core: 8
KC = N // P              # k-chunks: 64
ACH = 2048               # A stream chunk width (f32 -> 1 MiB per DMA)
NACH = N // ACH          # stream chunks per row-block: 4
GPC = ACH // (4 * P)     # transpose groups (of 4 tiles) per stream chunk: 4
MTG = 6                  # m-tiles in the big matmul group (PSUM banks)
HMT = 4                  # m-tiles per degree-collective half

F32 = mybir.dt.float32
BF16 = mybir.dt.bfloat16
F8 = mybir.dt.float8e4

_NC_CACHE = {}


def _build():
    nc = bacc.Bacc("TRN2", target_bir_lowering=False, debug=False, num_devices=NCORES)

    a_d = nc.dram_tensor("a", [R, N], F32, kind="ExternalInput")
    f_d = nc.dram_tensor("f", [N, D], F32, kind="ExternalInput")
    fres_d = nc.dram_tensor("fres", [R, D], F32, kind="ExternalInput")
    w_d = nc.dram_tensor("w", [D, D], F32, kind="ExternalInput")
    b_d = nc.dram_tensor("bias", [1, D], F32, kind="ExternalInput")
    out_d = nc.dram_tensor("out", [R, D], F32, kind="ExternalOutput")

    AX = mybir.AxisListType.X
    ALU = mybir.AluOpType
    ACT = mybir.ActivationFunctionType
    DR = mybir.MatmulPerfMode.DoubleRow

    with tile.TileContext(nc) as tc:
        with (
            tc.tile_pool(name="const", bufs=1) as constp,
            tc.tile_pool(name="deg", bufs=1) as degp,
            tc.tile_pool(name="astream", bufs=3) as astreamp,
            tc.tile_pool(name="small", bufs=2) as smallp,
            tc.tile_pool(name="atres", bufs=1) as atresp,
            tc.tile_pool(name="xp", bufs=1) as xpp,
            tc.tile_pool(name="f8p", bufs=1) as f8pool,
            tc.tile_pool(name="fstream", bufs=4) as fstreamp,
            tc.tile_pool(name="epi", bufs=2) as epip,
            tc.tile_pool(name="mfp", bufs=MT) as mfpool,
            tc.tile_pool(name="mft", bufs=2) as mftp,
            tc.tile_pool(name="psA", bufs=2, space="PSUM") as psA,      # transposes
            tc.tile_pool(name="psY", bufs=MTG, space="PSUM") as psY,    # Y + o accum
            tc.tile_pool(name="dram", bufs=1, space="DRAM") as dramp,
        ):
            # ---- constants ----
            identity = constp.tile([P, P], F32)
            make_identity(nc, identity[:])
            identity_f8 = constp.tile([P, P], F8)
            make_identity(nc, identity_f8[:])
            identity_bf = constp.tile([P, P], BF16)
            make_identity(nc, identity_bf[:])
            ones_row = constp.tile([1, P], BF16)
            nc.gpsimd.memset(ones_row[:], 1.0)
            bf_sb = constp.tile([1, D], F32)
            nc.sync.dma_start(bf_sb[:], b_d.ap())
            b_sb = constp.tile([1, D], BF16)
            nc.vector.tensor_copy(b_sb[:], bf_sb[:])
            w_sb = constp.tile([P, 4 * D], BF16)  # w chunk wc at [:, wc*D:(wc+1)*D]
            for wc in range(4):
                wf_ch = smallp.tile([P, D], F32, tag="wf")
                nc.sync.dma_start(wf_ch[:], w_d.ap()[wc * P : (wc + 1) * P, :])
                nc.vector.tensor_copy(w_sb[:, wc * D : (wc + 1) * D], wf_ch[:])

            # resident transposed-A store: (mt, kc) tile at col (mt*KC+kc)*P
            at_res = atresp.tile([P, MT * KC * P], F8)
            cin = [dramp.tile([HMT, P], F32, name=f"cin{h}") for h in range(2)]
            cout = [
                dramp.tile([NCORES * HMT, P], F32, name=f"cout{h}") for h in range(2)
            ]

            degree_sb = degp.tile([P, MT], F32)  # col mt = degree of rows mt*128..
            # ds64_sb[p, e*8 + mt] = 64 / sqrt(degree[global row e*1024+mt*128+p])
            ds64_sb = degp.tile([P, KC], F32)
            ds64_v = ds64_sb[:].rearrange("p (e c) -> p e c", e=NCORES)

            def issue_degree_half(h):
                """Transpose degree cols [h*4, h*4+4), AllGather them."""
                degTh_ps = psA.tile([HMT, P], F32, tag="trp", name=f"degT{h}")
                nc.tensor.matmul(
                    degTh_ps[:],
                    degree_sb[:, h * HMT : (h + 1) * HMT],
                    identity[:],
                )
                degTh_sb = smallp.tile([HMT, P], F32, tag="degT", name=f"degTs{h}")
                nc.vector.tensor_copy(degTh_sb[:], degTh_ps[:])
                nc.sync.dma_start(cin[h][:], degTh_sb[:])
                nc.gpsimd.collective_compute(
                    "AllGather",
                    ALU.bypass,
                    ins=[cin[h].opt()],
                    outs=[cout[h].opt()],
                    replica_groups=[list(range(NCORES))],
                )

            def consume_degree_half(h):
                """cout[h] row e*4+t = degree of rows [e*1024+(h*4+t)*128, +128)."""
                degall = smallp.tile(
                    [NCORES * HMT, P], F32, tag="degall", name=f"dga{h}"
                )
                nc.sync.dma_start(degall[:], cout[h][:])
                degallT_ps = psA.tile(
                    [P, NCORES * HMT], F32, tag="trp", name=f"dgaT{h}"
                )
                nc.tensor.transpose(
                    degallT_ps[:], degall[:], identity[: NCORES * HMT, : NCORES * HMT]
                )
                reciph = degp.tile([P, NCORES * HMT], F32, name=f"reciph{h}")
                nc.vector.reciprocal(reciph[:], degallT_ps[:])
                nc.scalar.activation(
                    ds64_v[:, :, h * HMT : (h + 1) * HMT],
                    reciph[:].rearrange("p (e c) -> p e c", e=NCORES),
                    ACT.Sqrt,
                    scale=4096.0,
                )

            # ---- merged pass: degree + transpose-all; half-collectives ----
            for mt in range(MT):
                dcols = smallp.tile([P, NACH], F32, tag="dcols")
                for c in range(NACH):
                    ach = astreamp.tile([P, ACH], F32, tag="ach")
                    nc.sync.dma_start(
                        ach[:], a_d.ap()[mt * P : (mt + 1) * P, c * ACH : (c + 1) * ACH]
                    )
                    ach8 = astreamp.tile([P, ACH], F8, tag="ach8", bufs=2)
                    nc.scalar.activation(
                        ach8[:], ach[:], ACT.Copy, accum_out=dcols[:, c : c + 1]
                    )
                    for g in range(GPC):
                        gk = c * GPC + g  # k-group index 0..15
                        trp = psA.tile([P, 4 * P], F32, tag="trp")
                        for q in range(4):
                            nc.tensor.matmul(
                                trp[:, q * P : (q + 1) * P],
                                ach8[:, (g * 4 + q) * P : (g * 4 + q + 1) * P],
                                identity_f8[:],
                            )
                        dst = at_res[
                            :, (mt * KC + gk * 4) * P : (mt * KC + gk * 4 + 4) * P
                        ]
                        nc.vector.tensor_copy(dst, trp[:])
                nc.vector.reduce_sum(degree_sb[:, mt : mt + 1], dcols[:], axis=AX)
                if mt == HMT - 1:
                    issue_degree_half(0)
            issue_degree_half(1)

            recip8 = degp.tile([P, MT], F32)
            nc.vector.reciprocal(recip8[:], degree_sb[:])
            # dsown64[p, mt] = 1 / (64 * sqrt(degree_own[mt*128 + p]))
            dsown64 = degp.tile([P, MT], F32)
            nc.scalar.activation(dsown64[:], recip8[:], ACT.Sqrt, scale=1.0 / 4096.0)

            # ---- f: stream + park as raw fp8 + scale to X'' + matmul, fully
            # interleaved per 2-chunk batch so every engine queue's order
            # matches data arrival. The first fch buffers take a fake WAW dep
            # on degree_sb so the f stream cannot start before the A stream
            # is done.
            f8raw = f8pool.tile([P, KC * D], F8)
            xp_sb = xpp.tile([P, KC * D], F8)  # chunk kc at [:, kc*D:(kc+1)*D]
            # sub-batch (h, e, s) = k-chunks e*8 + h*4 + s*2 + {0,1} = pair
            # pj = 4e + 2h + s
            f_blk = f_d.ap().rearrange(
                "(e h s c p) d -> h e s p c d", h=2, s=2, c=2, p=P
            )

            def mm_pair(y_ap, mt, pj, start, stop):
                """pj = global k-pair index (k-chunks 2*pj, 2*pj+1)."""
                base = (mt * KC + 2 * pj) * P
                at2 = at_res[:, base : base + 2 * P].rearrange(
                    "p (k m) -> p k m", k=2
                )
                xp2 = xp_sb[:, (2 * pj) * D : (2 * pj + 2) * D].rearrange(
                    "p (k n) -> p k n", k=2
                )
                nc.tensor.matmul(
                    y_ap, at2, xp2, start=start, stop=stop, perf_mode=DR
                )

            ys = [
                psY.tile([P, D], F32, tag="y", name=f"y{i}") for i in range(MTG)
            ]
            nfch = [0]

            def stream_half(h):
                consume_degree_half(h)
                for e in range(NCORES):
                    for s in range(2):
                        pj = 4 * e + 2 * h + s
                        fch = fstreamp.tile([P, 2 * D], F32, tag="fch")
                        if nfch[0] < 4:
                            nc.vector.tensor_copy(
                                fch[:, :1], degree_sb[:, MT - 1 : MT]
                            )
                        nfch[0] += 1
                        nc.sync.dma_start(
                            fch[:].rearrange("p (c d) -> p c d", c=2),
                            f_blk[h, e, s],
                        )
                        for c in range(2):
                            kc = 2 * pj + c
                            # split the f32->fp8 park across ScalarE and DVE
                            if c == 0:
                                nc.scalar.activation(
                                    f8raw[:, kc * D : (kc + 1) * D],
                                    fch[:, c * D : (c + 1) * D],
                                    ACT.Copy,
                                )
                            else:
                                nc.vector.tensor_copy(
                                    f8raw[:, kc * D : (kc + 1) * D],
                                    fch[:, c * D : (c + 1) * D],
                                )
                        for c in range(2):
                            kc = 2 * pj + c
                            nc.vector.tensor_scalar_mul(
                                xp_sb[:, kc * D : (kc + 1) * D],
                                f8raw[:, kc * D : (kc + 1) * D],
                                ds64_sb[:, kc : kc + 1],
                            )
                        for mi in range(MTG):
                            mm_pair(ys[mi][:], mi, pj, pj == 0, pj == KC // 2 - 1)

            stream_half(0)
            stream_half(1)

            # group 2: m-tiles 6,7 rotate into freed y slots
            ys2 = [
                psY.tile([P, D], F32, tag="y", name=f"y{MTG + i}")
                for i in range(MT - MTG)
            ]
            # epilogue part 1 for group 1: free the Y banks early
            mfs = []
            for mt in range(MTG):
                res = epip.tile([P, D], F32, tag="res")
                nc.sync.dma_start(res[:], fres_d.ap()[mt * P : (mt + 1) * P, :])
                mf = mfpool.tile([P, D], BF16, tag="mf", name=f"mf{mt}")
                nc.vector.scalar_tensor_tensor(
                    mf[:],
                    ys[mt][:],
                    dsown64[:, mt : mt + 1],
                    res[:],
                    op0=ALU.mult,
                    op1=ALU.add,
                )
                mfs.append(mf)
            for h in range(2):
                for e in range(NCORES):
                    for s in range(2):
                        pj = 4 * e + 2 * h + s
                        for i, mt in enumerate(range(MTG, MT)):
                            mm_pair(
                                ys2[i][:],
                                mt,
                                pj,
                                pj == 0,
                                pj == KC // 2 - 1,
                            )
            for i, mt in enumerate(range(MTG, MT)):
                res = epip.tile([P, D], F32, tag="res")
                nc.sync.dma_start(res[:], fres_d.ap()[mt * P : (mt + 1) * P, :])
                mf = mfpool.tile([P, D], BF16, tag="mf", name=f"mf{mt}")
                nc.vector.scalar_tensor_tensor(
                    mf[:],
                    ys2[i][:],
                    dsown64[:, mt : mt + 1],
                    res[:],
                    op0=ALU.mult,
                    op1=ALU.add,
                )
                mfs.append(mf)

            # epilogue part 2: out = relu(mf @ W + b), o accumulators rotate
            # through the freed psY slots
            for mt in range(MT):
                o_ps = psY.tile([P, D], F32, tag="y", name=f"o{mt}")
                for wc in range(4):
                    mfT_ps = psA.tile([P, P], F32, tag="trp")
                    nc.tensor.matmul(
                        mfT_ps[:], mfs[mt][:, wc * P : (wc + 1) * P], identity_bf[:]
                    )
                    mfT_sb = mftp.tile([P, P], BF16, tag="mfT")
                    nc.vector.tensor_copy(mfT_sb[:], mfT_ps[:])
                    nc.tensor.matmul(
                        o_ps[:],
                        mfT_sb[:],
                        w_sb[:, wc * D : (wc + 1) * D],
                        start=(wc == 0),
                        stop=False,
                    )
                nc.tensor.matmul(
                    o_ps[:], ones_row[:], b_sb[:], start=False, stop=True
                )
                osb = epip.tile([P, D], F32, tag="osb")
                nc.scalar.activation(osb[:], o_ps[:], ACT.Relu)
                nc.sync.dma_start(out_d.ap()[mt * P : (mt + 1) * P, :], osb[:])

    nc.compile()
    return nc


def _get_nc():
    if "nc" not in _NC_CACHE:
        _NC_CACHE["nc"] = _build()
    return _NC_CACHE["nc"]


def run(inputs, trace=False, trace_kwargs=None):
    """Run the SPMD kernel; returns (full_output, BassKernelResults)."""
    a = np.ascontiguousarray(np.asarray(inputs["adjacency_matrix"], dtype=np.float32))
    f = np.ascontiguousarray(np.asarray(inputs["feature"], dtype=np.float32))
    w = np.ascontiguousarray(np.asarray(inputs["W"], dtype=np.float32))
    b = np.ascontiguousarray(np.asarray(inputs["b"], dtype=np.float32)).reshape(1, D)

    nc = _get_nc()
    in_maps = []
    for d in range(NCORES):
        rows = slice(d * R, (d + 1) * R)
        in_maps.append({"a": a[rows], "f": f, "fres": f[rows], "w": w, "bias": b})
    res = bass_utils.run_bass_kernel_spmd(
        nc,
        in_maps,
        core_ids=list(range(NCORES)),
        trace=trace,
        **(trace_kwargs or {}),
    )
    out = np.concatenate([r["out"] for r in res.results], axis=0)
    return out, res


def kernel(**inputs):
    out, _ = run(inputs, trace=False)
    return out
